# revision 1
# baseline (speedup 1.0000x reference)
"""Trainium2 Bass kernel for nn_DiUT_Llama_65025804861930 (moe_routing).

kernel(**inputs) takes the FULL inputs of reference.setup_inputs() and
returns (moe, aux) exactly like reference.reference(**inputs).

Strategy (8 NeuronCores, two SPMD Bass kernels, no collectives):

  Kernel 1 (batch-pair sequence parallel): core c -> batch b=c//2, seq half
  r=c%2.  Each core computes the adaLN conditioning, q/k for its FULL batch
  (replicated within the pair, cheaper than a collective), feature-dim
  attention, the MoE-input activations xm (transposed) and router logits for
  its own 512 tokens.  Matmuls run in float32r (full PE rate) except the
  router logit matmul and the adaLN matmul which stay fp32 so the top-2
  expert selection matches the fp32 reference bit-stably.

  Host: L2-normalize logits over the sequence axis, softmax, top-2, build
  exact per-expert token lists (the "all-to-all dispatch" is host-mediated
  between the two kernels), and compute the aux load-balance scalar.

  Kernel 2 (expert parallel): core e holds expert e's W1/W2/W3 and its
  gathered tokens (padded to a uniform capacity).  Computes
  w * (sin(x W1) * (x W3)) W2 per token in float32r.

  Host: scatter-add the two expert contributions per token -> moe.
"""
import sys

sys.path.insert(0, "/opt/trn_rl_repo")

import numpy as np

import concourse.bass as bass
import concourse.tile as tile
from concourse import bacc, mybir
from concourse.bass_utils import run_bass_kernel_spmd
from concourse.masks import make_identity

F32 = mybir.dt.float32
F32R = mybir.dt.float32r
AF = mybir.ActivationFunctionType
OP = mybir.AluOpType
AX = mybir.AxisListType

B, S, D, E, H, TOPK = 4, 1024, 512, 8, 1536, 2
T = B * S
SB = 1024   # full batch seq per core (kernel 1)
SO = 512    # own token count per core
DT = D // 128
ST = SB // 128
TS = SO // 128
HT = H // 128
EPS = 1e-5


# ============================ kernel 1 ============================

def _build_k1():
    nc = bacc.Bacc("TRN2", target_bir_lowering=False, debug=False, num_devices=8)

    def din(name, shape):
        return nc.dram_tensor(name, shape, F32, kind="ExternalInput").ap()

    xb = din("xb", [SB, D])
    cb = din("cb", [D])
    W_ada = din("W_ada", [D, 6 * D])
    b_ada = din("b_ada", [6 * D])
    Wq = din("Wq", [D, D]); Wk = din("Wk", [D, D])
    Wv = din("Wv", [D, D]); Wo = din("Wo", [D, D])
    vec_names = ["an_w", "an_b", "fn_w", "fn_b", "qn_w", "qn_b", "kn_w", "kn_b"]
    vecs = {n: din(n, [D]) for n in vec_names}
    Wr = din("Wr", [D, 8])
    br = din("br", [8])

    xmT_out = nc.dram_tensor("xmT", [D, SO], F32, kind="ExternalOutput").ap()
    lgT_out = nc.dram_tensor("logitsT", [8, SO], F32, kind="ExternalOutput").ap()

    with tile.TileContext(nc) as tc:
        _k1_body(nc, tc, xb, cb, W_ada, b_ada, Wq, Wk, Wv, Wo, vecs, Wr, br,
                 xmT_out, lgT_out)
    nc.compile()
    return nc


def _k1_body(nc, tc, xb, cb, W_ada, b_ada, Wq, Wk, Wv, Wo, vecs, Wr, br,
             xmT_out, lgT_out):
    import contextlib
    with contextlib.ExitStack() as ctx:
        const = ctx.enter_context(tc.tile_pool(name="const", bufs=1))
        stat = ctx.enter_context(tc.tile_pool(name="stat", bufs=4))
        # PSUM budget (8 banks): mm x3 + tr x2 + tr32 x1 + sm x1 + lg x1
        ps = ctx.enter_context(tc.tile_pool(name="ps", bufs=3, space="PSUM"))
        pst = ctx.enter_context(tc.tile_pool(name="pst", bufs=2, space="PSUM"))
        pst32 = ctx.enter_context(tc.tile_pool(name="pst32", bufs=1, space="PSUM"))
        psm_pool = ctx.enter_context(tc.tile_pool(name="psm", bufs=1, space="PSUM"))
        psl_pool = ctx.enter_context(tc.tile_pool(name="psl", bufs=1, space="PSUM"))

        ident_32 = const.tile([128, 128], F32)
        make_identity(nc, ident_32)
        ident_r = const.tile([128, 128], F32R)
        nc.vector.tensor_copy(ident_r[:], ident_32[:])
        eps_t = const.tile([128, 1], F32)
        nc.vector.memset(eps_t[:], EPS)
        br_t = const.tile([8, 1], F32)
        nc.sync.dma_start(br_t[:], br.rearrange("(e o) -> e o", o=1))
        csil = const.tile([128, DT], F32)
        craw = const.tile([128, DT], F32)
        nc.sync.dma_start(craw[:], cb.rearrange("(dt p) -> p dt", p=128))
        nc.scalar.activation(csil[:], craw[:], AF.Silu)
        Wr_sb = const.tile([128, DT, 8], F32)
        nc.sync.dma_start(Wr_sb[:], Wr.rearrange("(dt p) e -> p dt e", p=128))

        bigbc = ctx.enter_context(tc.tile_pool(name="bigbc", bufs=1))
        bc = {}
        for n in ("A_msa", "B_msa", "A_mlp", "B_mlp", "g_msa",
                  "qn_w", "qn_b", "kn_w", "kn_b"):
            bc[n] = bigbc.tile([128, D], F32, tag=f"bc_{n}", name=f"bc_{n}")

        # ---------- setup scope: mods & modulate vectors ----------
        with tc.tile_pool(name="setup", bufs=1) as setup:
            vrow = {}
            for n in ("an_w", "an_b", "fn_w", "fn_b", "qn_w", "qn_b",
                      "kn_w", "kn_b"):
                t = setup.tile([1, D], F32, tag=f"vrow_{n}", name=f"vrow_{n}")
                nc.sync.dma_start(t[:], vecs[n].rearrange("(o d) -> o d", o=1))
                vrow[n] = t
            bada = setup.tile([1, 6, D], F32)
            nc.sync.dma_start(bada[:], b_ada.rearrange("(o j) -> o j", o=1))

            mods = setup.tile([1, 6, D], F32)
            wada = setup.tile([128, DT, 6 * D], F32)
            W_ada_re = W_ada.rearrange("(dt p) j -> dt p j", p=128)
            for dt in range(DT):
                nc.sync.dma_start(wada[:, dt, :], W_ada_re[dt])
            for j in range(6):
                pm = psm_pool.tile([1, D], F32, tag="sm")
                for dt in range(DT):
                    nc.tensor.matmul(pm[:], csil[:, dt:dt + 1],
                                     wada[:, dt, j * D:(j + 1) * D],
                                     start=(dt == 0), stop=(dt == DT - 1))
                nc.vector.tensor_add(mods[0:1, j, :], pm[:], bada[0:1, j, :])

            # mods rows: 0 sh_msa, 1 sc_msa, 2 g_msa, 3 sh_mlp, 4 sc_mlp, 5 g_mlp
            cvec = setup.tile([1, 5, D], F32)  # A_msa, B_msa, A_mlp, B_mlp, g_msa
            tmp1 = setup.tile([1, D], F32, tag="tmp1")
            for i, (wn, bn, sc_j, sh_j) in enumerate(
                    (("an_w", "an_b", 1, 0), ("fn_w", "fn_b", 4, 3))):
                nc.vector.tensor_scalar_add(tmp1[:], mods[0:1, sc_j, :], 1.0)
                nc.vector.tensor_mul(cvec[0:1, 2 * i, :], tmp1[:], vrow[wn][:])
                nc.vector.tensor_mul(tmp1[:], tmp1[:], vrow[bn][:])
                nc.vector.tensor_add(cvec[0:1, 2 * i + 1, :], tmp1[:],
                                     mods[0:1, sh_j, :])
            nc.vector.tensor_copy(cvec[0:1, 4, :], mods[0:1, 2, :])

            for i, n in enumerate(("A_msa", "B_msa", "A_mlp", "B_mlp", "g_msa")):
                nc.gpsimd.partition_broadcast(bc[n][:], cvec[0:1, i, :])
            for n in ("qn_w", "qn_b", "kn_w", "kn_b"):
                nc.gpsimd.partition_broadcast(bc[n][:], vrow[n][:])

        work = ctx.enter_context(tc.tile_pool(name="work", bufs=2))
        woP = ctx.enter_context(tc.tile_pool(name="woP", bufs=1))
        big = ctx.enter_context(tc.tile_pool(name="big", bufs=1))

        def load_w_r(pool, name, w):
            raw = work.tile([128, DT, D], F32, tag="wraw")
            nc.sync.dma_start(raw[:], w.rearrange("(dt p) n -> p dt n", p=128))
            t = pool.tile([128, DT, D], F32R, tag=f"wr_{name}", name=f"wr_{name}")
            nc.vector.tensor_copy(t[:], raw[:])
            return t

        wo_r = load_w_r(woP, "Wo", Wo)

        def ln_apply(dst, src, scale_bc, bias_bc, tag):
            """dst = LN(src) * scale_bc + bias_bc  (src (128, D) fp32/psum)."""
            st6 = stat.tile([128, 6], F32, tag=f"st6_{tag}")
            mv = stat.tile([128, 2], F32, tag=f"mv_{tag}")
            nc.vector.bn_stats(st6[:], src)
            nc.vector.bn_aggr(mv[:], st6[:])
            sd = stat.tile([128, 1], F32, tag=f"sd_{tag}")
            nc.scalar.activation(sd[:], mv[:, 1:2], AF.Sqrt, bias=eps_t[:])
            r = stat.tile([128, 1], F32, tag=f"r_{tag}")
            nc.vector.reciprocal(r[:], sd[:])
            nmr = stat.tile([128, 1], F32, tag=f"nmr_{tag}")
            nc.vector.tensor_scalar(nmr[:], mv[:, 0:1], r[:], -1.0,
                                    op0=OP.mult, op1=OP.mult)
            xh = work.tile([128, D], F32, tag="xh")
            nc.scalar.activation(xh[:], src, AF.Identity, bias=nmr[:], scale=r[:])
            nc.vector.tensor_mul(xh[:], xh[:], scale_bc[:])
            nc.vector.tensor_add(dst, xh[:], bias_bc[:])

        xb_re = xb.rearrange("(st p) d -> st p d", p=128)

        vT = big.tile([128, DT, SO], F32R, tag="vT")
        AT = big.tile([128, DT, D], F32R, tag="AT")
        attnT = big.tile([128, DT, SO], F32R, tag="attnT")
        xmT = big.tile([128, DT, SO], F32, tag="xmT")

        with tc.tile_pool(name="qk", bufs=1) as qk_pool:
            q_sb = qk_pool.tile([128, ST, D], F32R, tag="q_sb")
            k_sb = qk_pool.tile([128, ST, D], F32R, tag="k_sb")

            with tc.tile_pool(name="hph", bufs=1) as hph:
                with tc.tile_pool(name="wqkv", bufs=1) as wqkv:
                    wq_r = load_w_r(wqkv, "Wq", Wq)
                    wk_r = load_w_r(wqkv, "Wk", Wk)
                    wv_r = load_w_r(wqkv, "Wv", Wv)

                    hT = hph.tile([128, DT, SB], F32R)
                    for st in range(ST):
                        xt = work.tile([128, D], F32, tag="xt")
                        nc.sync.dma_start(xt[:], xb_re[st])
                        ht = work.tile([128, D], F32R, tag="ht")
                        ln_apply(ht[:], xt[:], bc["A_msa"], bc["B_msa"], "h")
                        for dt in range(DT):
                            pt = pst.tile([128, 128], F32R, tag="tr")
                            nc.tensor.transpose(
                                pt[:], ht[:, dt * 128:(dt + 1) * 128], ident_r[:])
                            nc.scalar.copy(hT[:, dt, st * 128:(st + 1) * 128],
                                           pt[:])

                    for (proj, wr_t, dst, swn, sbn) in (
                            ("q", wq_r, q_sb, "qn_w", "qn_b"),
                            ("k", wk_r, k_sb, "kn_w", "kn_b")):
                        for st in range(ST):
                            pq = ps.tile([128, D], F32, tag="mm")
                            for dt in range(DT):
                                nc.tensor.matmul(
                                    pq[:], hT[:, dt, st * 128:(st + 1) * 128],
                                    wr_t[:, dt, :],
                                    start=(dt == 0), stop=(dt == DT - 1))
                            ln_apply(dst[:, st, :], pq[:], bc[swn], bc[sbn], proj)

                    for et in range(DT):
                        pv = ps.tile([128, SO], F32, tag="mm")
                        for dt in range(DT):
                            nc.tensor.matmul(
                                pv[:], wv_r[:, dt, et * 128:(et + 1) * 128],
                                hT[:, dt, 0:SO],
                                start=(dt == 0), stop=(dt == DT - 1))
                        nc.scalar.copy(vT[:, et, :], pv[:])

            # scores + softmax + AT
            for dt in range(DT):
                psc = ps.tile([128, D], F32, tag="mm")
                for st in range(ST):
                    nc.tensor.matmul(psc[:], q_sb[:, st, dt * 128:(dt + 1) * 128],
                                     k_sb[:, st, :], start=(st == 0),
                                     stop=(st == ST - 1))
                mx = stat.tile([128, 1], F32, tag="mx")
                nc.vector.reduce_max(mx[:], psc[:], axis=AX.X)
                nmx = stat.tile([128, 1], F32, tag="nmx")
                nc.vector.tensor_scalar_mul(nmx[:], mx[:], -1.0)
                aun = work.tile([128, D], F32, tag="aun")
                asum = stat.tile([128, 1], F32, tag="asum")
                nc.scalar.activation(aun[:], psc[:], AF.Exp, bias=nmx[:],
                                     accum_out=asum[:])
                rinv = stat.tile([128, 1], F32, tag="rinv")
                nc.vector.reciprocal(rinv[:], asum[:])
                at_row = work.tile([128, D], F32R, tag="atrow")
                nc.vector.tensor_scalar_mul(at_row[:], aun[:], rinv[:])
                for et in range(DT):
                    pt = pst.tile([128, 128], F32R, tag="tr")
                    nc.tensor.transpose(pt[:], at_row[:, et * 128:(et + 1) * 128],
                                        ident_r[:])
                    nc.scalar.copy(AT[:, et, dt * 128:(dt + 1) * 128], pt[:])

        # attnT (own half)
        for dt in range(DT):
            pa = ps.tile([128, SO], F32, tag="mm")
            for et in range(DT):
                nc.tensor.matmul(pa[:], AT[:, et, dt * 128:(dt + 1) * 128],
                                 vT[:, et, :], start=(et == 0), stop=(et == DT - 1))
            nc.scalar.copy(attnT[:, dt, :], pa[:])

        # x2 = x_own + g*(attn@Wo); xm; xmT; logits
        xmT_re = xmT_out.rearrange("(dt p) s -> dt p s", p=128)
        for ts in range(TS):
            po = ps.tile([128, D], F32, tag="mm")
            for dt in range(DT):
                nc.tensor.matmul(po[:], attnT[:, dt, ts * 128:(ts + 1) * 128],
                                 wo_r[:, dt, :], start=(dt == 0),
                                 stop=(dt == DT - 1))
            xot = work.tile([128, D], F32, tag="xt")
            nc.sync.dma_start(xot[:], xb_re[ts])  # own half = first 4 s-tiles
            x2 = work.tile([128, D], F32, tag="x2")
            nc.vector.tensor_mul(x2[:], po[:], bc["g_msa"][:])
            nc.vector.tensor_add(x2[:], x2[:], xot[:])
            xm_t = work.tile([128, D], F32, tag="xm_t")
            ln_apply(xm_t[:], x2[:], bc["A_mlp"], bc["B_mlp"], "xm")
            for dt in range(DT):
                pt = pst32.tile([128, 128], F32, tag="tr32")
                nc.tensor.transpose(pt[:], xm_t[:, dt * 128:(dt + 1) * 128],
                                    ident_32[:])
                nc.scalar.copy(xmT[:, dt, ts * 128:(ts + 1) * 128], pt[:])
        for dt in range(DT):
            nc.sync.dma_start(xmT_re[dt], xmT[:, dt, :])

        pl = psl_pool.tile([8, SO], F32, tag="lg")
        for dt in range(DT):
            nc.tensor.matmul(pl[:], Wr_sb[:, dt, :], xmT[:, dt, :],
                             start=(dt == 0), stop=(dt == DT - 1))
        lg = work.tile([8, SO], F32, tag="lg")
        nc.vector.tensor_scalar(lg[:], pl[:], br_t[:], None, op0=OP.add)
        nc.sync.dma_start(lgT_out, lg[:])


# ============================ kernel 2 ============================

def _build_k2(C):
    assert C % 256 == 0
    chunks = []
    rem = C
    while rem >= 512:
        chunks.append(512)
        rem -= 512
    if rem:
        chunks.append(rem)

    nc = bacc.Bacc("TRN2", target_bir_lowering=False, debug=False, num_devices=8)
    W1 = nc.dram_tensor("W1", [D, H], F32, kind="ExternalInput").ap()
    W2 = nc.dram_tensor("W2", [H, D], F32, kind="ExternalInput").ap()
    W3 = nc.dram_tensor("W3", [D, H], F32, kind="ExternalInput").ap()
    xg = nc.dram_tensor("xg", [D, C], F32, kind="ExternalInput").ap()
    wv = nc.dram_tensor("wv", [C], F32, kind="ExternalInput").ap()
    yT_out = nc.dram_tensor("yT", [D, C], F32, kind="ExternalOutput").ap()

    with tile.TileContext(nc) as tc:
        _k2_body(nc, tc, W1, W2, W3, xg, wv, yT_out, C, chunks)
    nc.compile()
    return nc


def _k2_body(nc, tc, W1, W2, W3, xg, wv, yT_out, C, chunks):
    import contextlib
    with contextlib.ExitStack() as ctx:
        wpool = ctx.enter_context(tc.tile_pool(name="wpool", bufs=1))
        stream = ctx.enter_context(tc.tile_pool(name="stream", bufs=2))
        gbuf = ctx.enter_context(tc.tile_pool(name="gbuf", bufs=2))
        obuf = ctx.enter_context(tc.tile_pool(name="obuf", bufs=2))
        ps13 = ctx.enter_context(tc.tile_pool(name="ps13", bufs=4, space="PSUM"))
        psy = ctx.enter_context(tc.tile_pool(name="psy", bufs=4, space="PSUM"))

        w1r = wpool.tile([128, DT, H], F32R)
        w3r = wpool.tile([128, DT, H], F32R)
        w2r = wpool.tile([128, HT, D], F32R)
        xgr = wpool.tile([128, DT, C], F32R)
        W1_re = W1.rearrange("(dt p) h -> dt p h", p=128)
        W3_re = W3.rearrange("(dt p) h -> dt p h", p=128)
        W2_re = W2.rearrange("(ht p) d -> ht p d", p=128)
        xg_re = xg.rearrange("(dt p) t -> dt p t", p=128)
        for dt in range(DT):
            raw = stream.tile([128, H], F32, tag="wraw")
            nc.sync.dma_start(raw[:], W1_re[dt])
            nc.vector.tensor_copy(w1r[:, dt, :], raw[:])
            raw = stream.tile([128, H], F32, tag="wraw")
            nc.sync.dma_start(raw[:], W3_re[dt])
            nc.vector.tensor_copy(w3r[:, dt, :], raw[:])
            raw = stream.tile([128, C], F32, tag="xraw")
            nc.sync.dma_start(raw[:], xg_re[dt])
            nc.vector.tensor_copy(xgr[:, dt, :], raw[:])
        for ht in range(HT):
            raw = stream.tile([128, D], F32, tag="w2raw")
            nc.sync.dma_start(raw[:], W2_re[ht])
            nc.vector.tensor_copy(w2r[:, ht, :], raw[:])

        wrow = wpool.tile([1, C], F32)
        nc.sync.dma_start(wrow[:], wv.rearrange("(o t) -> o t", o=1))
        wbc = wpool.tile([128, C], F32)
        nc.gpsimd.partition_broadcast(wbc[:], wrow[:])

        yT_re = yT_out.rearrange("(dp p) t -> dp p t", p=128)

        c0 = 0
        for cw in chunks:
            csl = slice(c0, c0 + cw)
            gT = gbuf.tile([128, HT, 512], F32R, tag="gT")
            for ht in range(HT):
                p1 = ps13.tile([128, 512], F32, tag="h13")
                p3 = ps13.tile([128, 512], F32, tag="h13")
                for dt in range(DT):
                    nc.tensor.matmul(p1[:, :cw],
                                     w1r[:, dt, ht * 128:(ht + 1) * 128],
                                     xgr[:, dt, csl],
                                     start=(dt == 0), stop=(dt == DT - 1))
                for dt in range(DT):
                    nc.tensor.matmul(p3[:, :cw],
                                     w3r[:, dt, ht * 128:(ht + 1) * 128],
                                     xgr[:, dt, csl],
                                     start=(dt == 0), stop=(dt == DT - 1))
                sing = gbuf.tile([128, 512], F32, tag="sing")
                nc.scalar.activation(sing[:, :cw], p1[:, :cw], AF.Sin)
                nc.vector.tensor_mul(gT[:, ht, :cw], sing[:, :cw], p3[:, :cw])

            for dp in range(DT):
                py = psy.tile([128, 512], F32, tag="y")
                for ht in range(HT):
                    nc.tensor.matmul(py[:, :cw],
                                     w2r[:, ht, dp * 128:(dp + 1) * 128],
                                     gT[:, ht, :cw],
                                     start=(ht == 0), stop=(ht == HT - 1))
                yw = obuf.tile([128, 512], F32, tag="yw")
                nc.vector.tensor_mul(yw[:, :cw], py[:, :cw], wbc[:, csl])
                nc.sync.dma_start(yT_re[dp][:, csl], yw[:, :cw])
            c0 += cw


# ============================ host glue ============================

_K1_CACHE = {}
_K2_CACHE = {}


def _get_k1():
    if "k1" not in _K1_CACHE:
        _K1_CACHE["k1"] = _build_k1()
    return _K1_CACHE["k1"]


def _get_k2(C):
    if C not in _K2_CACHE:
        _K2_CACHE[C] = _build_k2(C)
    return _K2_CACHE[C]


def kernel(x, c, W_ada, b_ada, Wq, Wk, Wv, Wo, qn_w, qn_b, kn_w, kn_b,
           an_w, an_b, fn_w, fn_b, Wr, br, W1, W2, W3):
    f32 = lambda a: np.ascontiguousarray(np.asarray(a, dtype=np.float32))
    x = f32(x); c = f32(c)
    shared = dict(
        W_ada=f32(W_ada), b_ada=f32(b_ada), Wq=f32(Wq), Wk=f32(Wk),
        Wv=f32(Wv), Wo=f32(Wo), an_w=f32(an_w), an_b=f32(an_b),
        fn_w=f32(fn_w), fn_b=f32(fn_b), qn_w=f32(qn_w), qn_b=f32(qn_b),
        kn_w=f32(kn_w), kn_b=f32(kn_b), Wr=f32(Wr), br=f32(br))
    W1 = f32(W1); W2 = f32(W2); W3 = f32(W3)

    # ---- kernel 1: attention + router logits ----
    nc1 = _get_k1()
    in_maps = []
    for core in range(8):
        b, r = core // 2, core % 2
        xb = np.roll(x[b], -r * 512, axis=0) if r else x[b]
        in_maps.append(dict(xb=np.ascontiguousarray(xb),
                            cb=np.ascontiguousarray(c[b]), **shared))
    res1 = run_bass_kernel_spmd(nc1, in_maps, core_ids=list(range(8)))

    xmT_all = np.empty((D, T), np.float32)       # feature-major xm
    logits = np.empty((T, E), np.float32)
    for core in range(8):
        sl = slice(core * SO, (core + 1) * SO)
        xmT_all[:, sl] = res1.results[core]["xmT"]
        logits[sl] = res1.results[core]["logitsT"].T

    # ---- host router: mirror reference ops in fp32 ----
    lg = logits.reshape(B, S, E)
    nrm = np.sqrt(np.sum(lg * lg, axis=1, keepdims=True))
    ln_ = lg / np.maximum(nrm, np.float32(1e-12))
    z = ln_ - ln_.max(-1, keepdims=True)
    p = np.exp(z)
    p = p / p.sum(-1, keepdims=True)             # probs (B,S,E) fp32

    avg = p.mean(axis=0)                          # (S,E)
    aux = np.sum((np.float32(1.0 / E) - avg) ** 2, dtype=np.float32)

    pf = p.reshape(T, E)
    idx = np.argsort(-pf, axis=-1, kind="stable")[:, :TOPK]
    vals = np.take_along_axis(pf, idx, axis=-1)

    # ---- dispatch: exact per-expert token lists ----
    lists, ws = [], []
    for e in range(E):
        sel = np.where((idx == e).any(axis=1))[0]
        slot = (idx[sel] == e).argmax(axis=1)
        w = np.take_along_axis(vals[sel], slot[:, None], axis=1)[:, 0]
        lists.append(sel)
        ws.append(w.astype(np.float32))
    counts = [len(l) for l in lists]
    C = max(256, -(-max(counts) // 256) * 256)

    # ---- kernel 2: expert FFN ----
    nc2 = _get_k2(C)
    in_maps2 = []
    for e in range(E):
        xg = np.zeros((D, C), np.float32)
        xg[:, :counts[e]] = xmT_all[:, lists[e]]
        wv = np.zeros((C,), np.float32)
        wv[:counts[e]] = ws[e]
        in_maps2.append(dict(W1=W1[e], W2=W2[e], W3=W3[e], xg=xg, wv=wv))
    res2 = run_bass_kernel_spmd(nc2, in_maps2, core_ids=list(range(8)))

    # ---- combine ----
    moe = np.zeros((T, D), np.float32)
    for e in range(E):
        yT = res2.results[e]["yT"]
        moe[lists[e]] += yT[:, :counts[e]].T
    moe = moe.reshape(B, S, D)

    return moe, np.float32(aux)
